# revision 13
# baseline (speedup 1.0000x reference)
"""GRU kernel for Trainium2 (Bass/Tile), 8-core batch-parallel.

Problem: x [T=2048, B=64, D=256] fp32, h0 [64, 512], Wz/Wr/Wh [768, 512],
bz/br/bh [512]. Returns hidden history [T, 64, 512] fp32.

Strategy:
  - Data-parallel over batch: core c handles batch rows c*8:(c+1)*8.
  - All on-device activations live in "transposed" layout: hidden dim on the
    128 partitions (4 column-groups of 8 batch cols), batch on the free dim.
  - Per timestep, the 3 gate matmuls run with the weight tile stationary
    (lhsT = W[k,m] 128x128 bf16) and hT streaming as an 8-column rhs.
  - The x-dependent part of all gates (x_t @ W[:D] + b) is batched over a
    64-step time tile as dense N=512 matmuls, cast to bf16 in SBUF, and
    injected into each step's PSUM accumulation via an identity matmul.
  - h is carried purely in bf16: the blend writes the hist tile directly
    (hist slice doubles as next step's matmul rhs), halving DVE work.
    The (1-z)*h term uses scalar_tensor_tensor: vbar=(z-1)*h; h'=u-vbar.
  - All 16 r-matmuls are emitted before the z-matmuls so sigmoid(r) fires
    as early as possible; z fills the PE while sigmoid(r)/r*h run.
  - xT tiles for the x-part are DMA'd one full 64-step tile ahead so the
    batched x-part matmuls never stall the PE queue on a DMA semaphore.
  - Host pre-transposes x to xT bf16 and post-transposes the output, so the
    device never transposes anything.
"""

import os
import sys

for _p in ("/opt/trn_rl_repo", os.path.expanduser("~/.axon_site/_ro/trn_rl_repo")):
    if os.path.isdir(_p) and _p not in sys.path:
        sys.path.insert(0, _p)

import numpy as np
import ml_dtypes

T, B, D, H = 2048, 64, 256, 512
NCORES = 8
BC = B // NCORES              # 8 batch rows per core
G = H // 128                  # 4 column-groups of the hidden dim
KH = H // 128                 # 4 contraction chunks for the h-part
KX = D // 128                 # 2 contraction chunks for the x-part
TT = 64                       # timesteps per loop iteration
BF16 = ml_dtypes.bfloat16


def _prep_w(w):
    # W [768, 512] -> [128, 6*512] bf16; col = k*512 + m*128 + j holds W[k*128+p, m*128+j]
    return np.ascontiguousarray(
        w.reshape(6, 128, 4, 128).transpose(1, 0, 2, 3).reshape(128, 3072)
    ).astype(BF16)


def _build_program(n_tiles):
    import concourse.bass as bass
    import concourse.tile as tile
    from concourse import bacc, mybir

    fp32 = mybir.dt.float32
    bf16 = mybir.dt.bfloat16
    Tn = n_tiles * TT

    nc = bacc.Bacc(
        "TRN2",
        target_bir_lowering=False,
        debug=False,
        enable_asserts=False,
        num_devices=NCORES,
    )

    assert n_tiles % 2 == 0, "loop body processes two tiles"
    # two tiles of zero padding at the end for the final (dead) prefetches
    xT_d = nc.dram_tensor("xT", [D, (Tn + 2 * TT) * BC], bf16, kind="ExternalInput")
    h0T_d = nc.dram_tensor("h0T", [128, G * BC], bf16, kind="ExternalInput")
    w_d = {
        g: nc.dram_tensor(f"W{g}", [128, 3072], bf16, kind="ExternalInput")
        for g in "zrh"
    }
    b_d = nc.dram_tensor("bT", [128, 12], fp32, kind="ExternalInput")
    id_d = nc.dram_tensor("ident", [128, 128], bf16, kind="ExternalInput")
    hist_d = nc.dram_tensor("histT", [128, Tn * G * BC], bf16, kind="ExternalOutput")

    from contextlib import ExitStack

    with tile.TileContext(nc) as tc, ExitStack() as ctx:
        persist = ctx.enter_context(tc.tile_pool(name="persist", bufs=1))
        wsb = {
            g: persist.tile([128, 3072], bf16, tag=f"W{g}", name=f"W{g}sb")
            for g in "zrh"
        }
        bsb = persist.tile([128, 12], fp32, tag="bT")
        ident = persist.tile([128, 128], bf16, tag="ident")
        h_cb = persist.tile([128, G * BC], bf16, tag="h_carry_b")   # h0 only

        for g in "zrh":
            nc.sync.dma_start(wsb[g][:], w_d[g].ap()[:])
        nc.sync.dma_start(bsb[:], b_d.ap()[:])
        nc.sync.dma_start(ident[:], id_d.ap()[:])
        nc.sync.dma_start(h_cb[:], h0T_d.ap()[:])

        xg_pool = ctx.enter_context(tc.tile_pool(name="xg", bufs=2))
        hist_pool = ctx.enter_context(tc.tile_pool(name="hist", bufs=2))
        sm_pool = ctx.enter_context(tc.tile_pool(name="small", bufs=3))
        ps_r = ctx.enter_context(tc.tile_pool(name="ps_r", bufs=2, space="PSUM"))
        ps_zr = ctx.enter_context(tc.tile_pool(name="ps_zr", bufs=2, space="PSUM"))
        ps_c = ctx.enter_context(tc.tile_pool(name="ps_c", bufs=2, space="PSUM"))
        ps_xg = ctx.enter_context(tc.tile_pool(name="ps_xg", bufs=2, space="PSUM"))

        xga = persist.tile([128, 12 * TT * BC], bf16, tag="xga")
        xgb = persist.tile([128, 12 * TT * BC], bf16, tag="xgb")
        # double-buffered persistent xT staging (one full tile ahead of use)
        xtA = [
            persist.tile([128, TT * BC], bf16, tag=f"xtA{k}", name=f"xtA{k}")
            for k in range(KX)
        ]
        xtB = [
            persist.tile([128, TT * BC], bf16, tag=f"xtB{k}", name=f"xtB{k}")
            for k in range(KX)
        ]

        def wtile(g, k, m):
            # lhsT tile for gate g, contraction chunk k (0,1=x-part, 2..5=h-part),
            # output chunk m
            return wsb[g][:, k * 512 + m * 128 : k * 512 + (m + 1) * 128]

        def emit_xfill_load(xts, xt_col_start):
            """DMA the xT slice for one future tile into persist tiles xts."""
            for k in range(KX):
                nc.sync.dma_start(
                    xts[k][:],
                    xT_d.ap()[
                        k * 128 : (k + 1) * 128,
                        bass.DynSlice(xt_col_start, TT * BC)
                        if not isinstance(xt_col_start, int)
                        else slice(xt_col_start, xt_col_start + TT * BC),
                    ],
                )

        def emit_xfill_mm(xg_t, xts, m12):
            """One gate-chunk of the batched x-part into xg_t (+bias, bf16)."""
            gate = "zrh"[m12 // 4]
            m = m12 % 4
            pxg = ps_xg.tile([128, TT * BC], fp32, tag="ps_xg", name="pxg")
            for k in range(KX):
                nc.tensor.matmul(
                    pxg[:],
                    wtile(gate, k, m),
                    xts[k][:],
                    start=(k == 0),
                    stop=(k == KX - 1),
                )
            nc.scalar.activation(
                xg_t[:, m12 * TT * BC : (m12 + 1) * TT * BC],
                pxg[:],
                mybir.ActivationFunctionType.Identity,
                bias=bsb[:, m12 : m12 + 1],
            )

        def recurrence(xg_use, xg_fill, xt_use, xt_load, load_col_start,
                       hist_col_start):
            """TT steps using xg_use; interleave x-part fill of xg_fill from
            xt_use (DMA'd one recurrence earlier); prefetch the next tile's
            xT into xt_load at load_col_start."""
            xg3 = xg_use[:].rearrange("p (m s) -> p m s", m=12)
            hist = hist_pool.tile([128, TT * G * BC], bf16, tag="hist", name="hist")
            for s in range(TT):
                h_prev = h_cb[:] if s == 0 else hist[:, (s - 1) * 32 : s * 32]

                # padded to a full 2KB PSUM bank each, so pool slots never
                # share a bank (bank-overlap tracking would serialize ACT
                # reads of step s behind PE writes of step s+1)
                pr = ps_r.tile(
                    [128, 32], fp32, tag="ps_r", name="pr", padded_shape=[128, 512]
                )
                pz = ps_zr.tile(
                    [128, 32], fp32, tag="ps_z", name="pz", padded_shape=[128, 512]
                )
                pc = ps_c.tile(
                    [128, 32], fp32, tag="ps_c", name="pc", padded_shape=[128, 512]
                )
                # inject the precomputed x-parts (+bias)
                nc.tensor.matmul(
                    pr[:], ident[:], xg3[:, 4:8, s * BC : (s + 1) * BC],
                    start=True, stop=False, skip_group_check=True,
                )
                nc.tensor.matmul(
                    pz[:], ident[:], xg3[:, 0:4, s * BC : (s + 1) * BC],
                    start=True, stop=False, skip_group_check=True,
                )
                nc.tensor.matmul(
                    pc[:], ident[:], xg3[:, 8:12, s * BC : (s + 1) * BC],
                    start=True, stop=False, skip_group_check=True,
                )
                # r gate first: its 16 matmuls are the head of the serial chain
                r_sb = sm_pool.tile([128, 32], fp32, tag="r_sb", name="r_sb")
                rh_b = sm_pool.tile([128, 32], bf16, tag="rh_b", name="rh_b")
                for k in range(KH):
                    for m in range(4):
                        nc.tensor.matmul(
                            pr[:, m * 8 : (m + 1) * 8],
                            wtile("r", 2 + k, m),
                            h_prev[:, k * 8 : (k + 1) * 8],
                            start=False, stop=(k == KH - 1 and m == 3),
                            skip_group_check=True,
                        )
                nc.scalar.activation(
                    r_sb[:], pr[:], mybir.ActivationFunctionType.Sigmoid
                )
                # z gate fills the PE while sigmoid(r) and r*h run
                z_sb = sm_pool.tile([128, 32], fp32, tag="z_sb", name="z_sb")
                for k in range(KH):
                    for m in range(4):
                        nc.tensor.matmul(
                            pz[:, m * 8 : (m + 1) * 8],
                            wtile("z", 2 + k, m),
                            h_prev[:, k * 8 : (k + 1) * 8],
                            start=False, stop=(k == KH - 1 and m == 3),
                            skip_group_check=True,
                        )
                nc.scalar.activation(
                    z_sb[:], pz[:], mybir.ActivationFunctionType.Sigmoid
                )
                nc.vector.tensor_mul(rh_b[:], r_sb[:], h_prev[:])
                # candidate
                for k in range(KH):
                    for m in range(4):
                        nc.tensor.matmul(
                            pc[:, m * 8 : (m + 1) * 8],
                            wtile("h", 2 + k, m),
                            rh_b[:, k * 8 : (k + 1) * 8],
                            start=False, stop=(k == KH - 1 and m == 3),
                            skip_group_check=True,
                        )
                # vbar = (z-1)*h_prev, off the critical path
                vb_sb = sm_pool.tile([128, 32], fp32, tag="vb_sb", name="vb_sb")
                nc.vector.scalar_tensor_tensor(
                    vb_sb[:], z_sb[:], 1.0, h_prev[:],
                    mybir.AluOpType.subtract, mybir.AluOpType.mult,
                )
                c_sb = sm_pool.tile([128, 32], fp32, tag="c_sb", name="c_sb")
                nc.scalar.activation(
                    c_sb[:], pc[:], mybir.ActivationFunctionType.Tanh
                )
                # h_new = z*c - (z-1)*h, written straight into hist as bf16;
                # the hist slice doubles as next step's matmul rhs
                u_sb = sm_pool.tile([128, 32], fp32, tag="u_sb", name="u_sb")
                nc.vector.tensor_mul(u_sb[:], z_sb[:], c_sb[:])
                nc.vector.tensor_tensor(
                    hist[:, s * 32 : (s + 1) * 32], u_sb[:], vb_sb[:],
                    mybir.AluOpType.subtract,
                )
                if s == TT - 1:
                    # refresh the persistent carry for the next recurrence
                    nc.vector.tensor_tensor(
                        h_cb[:], u_sb[:], vb_sb[:], mybir.AluOpType.subtract
                    )

                # interleave the next tile's x-part work into PE/ACT gaps
                # (emitted at end-of-step so the ACT cast queues after tanh)
                if xg_fill is not None:
                    if s == 0:
                        emit_xfill_load(xt_load, load_col_start)
                    if s % 5 == 1 and s // 5 < 12:
                        emit_xfill_mm(xg_fill, xt_use, s // 5)

            nc.sync.dma_start(
                hist_d.ap()[
                    :,
                    bass.DynSlice(hist_col_start, TT * G * BC)
                    if not isinstance(hist_col_start, int)
                    else slice(hist_col_start, hist_col_start + TT * G * BC),
                ],
                hist[:],
            )

        # prologue: fill xga for tile 0 via xtB, then stage tile 1 in xtB
        emit_xfill_load(xtB, 0)
        for m12 in range(12):
            emit_xfill_mm(xga, xtB, m12)
        emit_xfill_load(xtB, TT * BC)

        CPB = TT * BC  # xT cols per tile
        HPB = TT * G * BC  # hist cols per tile
        with tc.For_i(
            0, n_tiles // 2, 1,
            hint_engines=tuple(mybir.ALL_ENGINES),
        ) as i:
            # sub-tile 2i: consume xga; fill xgb from xtB (tile 2i+1);
            # prefetch xT of tile 2i+2 into xtA
            recurrence(
                xga, xgb, xtB, xtA, i * (2 * CPB) + 2 * CPB, i * (2 * HPB)
            )
            # sub-tile 2i+1: consume xgb; fill xga from xtA (tile 2i+2);
            # prefetch xT of tile 2i+3 into xtB
            recurrence(
                xgb, xga, xtA, xtB, i * (2 * CPB) + 3 * CPB, i * (2 * HPB) + HPB
            )

    nc.compile()
    return nc


def _run(inputs, n_tiles=T // TT, trace=False):
    from concourse.bass_utils import run_bass_kernel_spmd

    x = np.asarray(inputs["x"], dtype=np.float32)
    h0 = np.asarray(inputs["h0"], dtype=np.float32)
    Tn = n_tiles * TT
    x = x[:Tn]

    ws = {g: _prep_w(np.asarray(inputs[f"W{g}"], dtype=np.float32)) for g in "zrh"}
    bT = np.ascontiguousarray(
        np.stack(
            [np.asarray(inputs[f"b{g}"], dtype=np.float32).reshape(4, 128).T for g in "zrh"],
            axis=1,
        ).reshape(128, 12)
    )
    ident = np.eye(128, dtype=np.float32).astype(BF16)
    xT_all = x.astype(BF16).transpose(2, 0, 1)  # [D, Tn, B]

    in_maps = []
    for c in range(NCORES):
        sl = slice(c * BC, (c + 1) * BC)
        xT = np.zeros((D, (Tn + 2 * TT) * BC), dtype=BF16)
        xT[:, : Tn * BC] = xT_all[:, :, sl].reshape(D, Tn * BC)
        h0T = np.ascontiguousarray(
            h0[sl].reshape(BC, G, 128).transpose(2, 1, 0).reshape(128, G * BC)
        ).astype(BF16)
        in_maps.append(
            {
                "xT": xT,
                "h0T": h0T,
                "Wz": ws["z"], "Wr": ws["r"], "Wh": ws["h"],
                "bT": bT,
                "ident": ident,
            }
        )

    nc = _build_program(n_tiles)
    res = run_bass_kernel_spmd(nc, in_maps, core_ids=list(range(NCORES)), trace=trace)

    out = np.empty((Tn, B, H), dtype=np.float32)
    for c in range(NCORES):
        histT = res.results[c]["histT"]  # [128, Tn*G*BC] bf16
        out[:, c * BC : (c + 1) * BC, :] = (
            histT.reshape(128, Tn, G, BC).transpose(1, 3, 2, 0).reshape(Tn, BC, H)
            .astype(np.float32)
        )
    return out, res


def kernel(**inputs):
    out, _ = _run(inputs)
    return out


# revision 15
# speedup vs baseline: 1.1874x; 1.1874x over previous
"""GRU kernel for Trainium2 (Bass/Tile), 8-core batch-parallel.

Problem: x [T=2048, B=64, D=256] fp32, h0 [64, 512], Wz/Wr/Wh [768, 512],
bz/br/bh [512]. Returns hidden history [T, 64, 512] fp32.

Strategy:
  - Data-parallel over batch: core c handles batch rows c*8:(c+1)*8.
  - All on-device activations live in "transposed" layout: hidden dim on the
    128 partitions (4 column-groups of 8 batch cols), batch on the free dim.
  - Per timestep, the 3 gate matmuls run with the weight tile stationary
    (lhsT = W[k,m] 128x128 bf16) and hT streaming as an 8-column rhs.
  - The x-dependent part of all gates (x_t @ W[:D] + b) is batched over a
    64-step time tile as dense N=512 matmuls, cast to bf16 in SBUF, and
    injected into each step's PSUM accumulation via an identity matmul.
  - h is carried purely in bf16: the blend writes the hist tile directly
    (hist slice doubles as next step's matmul rhs), halving DVE work.
    The (1-z)*h term uses scalar_tensor_tensor: vbar=(z-1)*h; h'=u-vbar.
  - All 16 r-matmuls are emitted before the z-matmuls so sigmoid(r) fires
    as early as possible; z fills the PE while sigmoid(r)/r*h run.
  - xT tiles for the x-part are DMA'd one full 64-step tile ahead so the
    batched x-part matmuls never stall the PE queue on a DMA semaphore.
  - Host pre-transposes x to xT bf16 and post-transposes the output, so the
    device never transposes anything.
"""

import os
import sys

for _p in ("/opt/trn_rl_repo", os.path.expanduser("~/.axon_site/_ro/trn_rl_repo")):
    if os.path.isdir(_p) and _p not in sys.path:
        sys.path.insert(0, _p)

import numpy as np
import ml_dtypes

T, B, D, H = 2048, 64, 256, 512
NCORES = 8
BC = B // NCORES              # 8 batch rows per core
G = H // 128                  # 4 column-groups of the hidden dim
KH = H // 128                 # 4 contraction chunks for the h-part
KX = D // 128                 # 2 contraction chunks for the x-part
TT = 64                       # timesteps per loop iteration
BF16 = ml_dtypes.bfloat16


def _prep_w(w):
    # W [768, 512] -> [128, 6*512] bf16; col = k*512 + m*128 + j holds W[k*128+p, m*128+j]
    return np.ascontiguousarray(
        w.reshape(6, 128, 4, 128).transpose(1, 0, 2, 3).reshape(128, 3072)
    ).astype(BF16)


def _build_program(n_tiles):
    import concourse.bass as bass
    import concourse.tile as tile
    from concourse import bacc, mybir

    fp32 = mybir.dt.float32
    bf16 = mybir.dt.bfloat16
    Tn = n_tiles * TT

    nc = bacc.Bacc(
        "TRN2",
        target_bir_lowering=False,
        debug=False,
        enable_asserts=False,
        num_devices=NCORES,
    )

    assert n_tiles % 2 == 0, "loop body processes two tiles"
    # two tiles of zero padding at the end for the final (dead) prefetches
    xT_d = nc.dram_tensor("xT", [D, (Tn + 2 * TT) * BC], bf16, kind="ExternalInput")
    h0T_d = nc.dram_tensor("h0T", [128, G * BC], bf16, kind="ExternalInput")
    w_d = {
        g: nc.dram_tensor(f"W{g}", [128, 3072], bf16, kind="ExternalInput")
        for g in "zrh"
    }
    b_d = nc.dram_tensor("bT", [128, 12], fp32, kind="ExternalInput")
    id_d = nc.dram_tensor("ident", [128, 128], bf16, kind="ExternalInput")
    hist_d = nc.dram_tensor("histT", [128, Tn * G * BC], bf16, kind="ExternalOutput")

    from contextlib import ExitStack

    with tile.TileContext(nc) as tc, ExitStack() as ctx:
        persist = ctx.enter_context(tc.tile_pool(name="persist", bufs=1))
        wsb = {
            g: persist.tile([128, 3072], bf16, tag=f"W{g}", name=f"W{g}sb")
            for g in "zrh"
        }
        bsb = persist.tile([128, 12], fp32, tag="bT")
        ident = persist.tile([128, 128], bf16, tag="ident")
        h_cb = persist.tile([128, G * BC], bf16, tag="h_carry_b")   # h0 only

        for g in "zrh":
            nc.sync.dma_start(wsb[g][:], w_d[g].ap()[:])
        nc.sync.dma_start(bsb[:], b_d.ap()[:])
        nc.sync.dma_start(ident[:], id_d.ap()[:])
        nc.sync.dma_start(h_cb[:], h0T_d.ap()[:])

        xg_pool = ctx.enter_context(tc.tile_pool(name="xg", bufs=2))
        hist_pool = ctx.enter_context(tc.tile_pool(name="hist", bufs=2))
        sm_pool = ctx.enter_context(tc.tile_pool(name="small", bufs=3))
        ps_r = ctx.enter_context(tc.tile_pool(name="ps_r", bufs=2, space="PSUM"))
        ps_zr = ctx.enter_context(tc.tile_pool(name="ps_zr", bufs=2, space="PSUM"))
        ps_c = ctx.enter_context(tc.tile_pool(name="ps_c", bufs=2, space="PSUM"))
        ps_xg = ctx.enter_context(tc.tile_pool(name="ps_xg", bufs=2, space="PSUM"))

        xga = persist.tile([128, 12 * TT * BC], bf16, tag="xga")
        xgb = persist.tile([128, 12 * TT * BC], bf16, tag="xgb")
        # double-buffered persistent xT staging (one full tile ahead of use)
        xtA = [
            persist.tile([128, TT * BC], bf16, tag=f"xtA{k}", name=f"xtA{k}")
            for k in range(KX)
        ]
        xtB = [
            persist.tile([128, TT * BC], bf16, tag=f"xtB{k}", name=f"xtB{k}")
            for k in range(KX)
        ]

        def wtile(g, k, m):
            # lhsT tile for gate g, contraction chunk k (0,1=x-part, 2..5=h-part),
            # output chunk m
            return wsb[g][:, k * 512 + m * 128 : k * 512 + (m + 1) * 128]

        def emit_xfill_load(xts, xt_col_start):
            """DMA the xT slice for one future tile into persist tiles xts."""
            for k in range(KX):
                nc.sync.dma_start(
                    xts[k][:],
                    xT_d.ap()[
                        k * 128 : (k + 1) * 128,
                        bass.DynSlice(xt_col_start, TT * BC)
                        if not isinstance(xt_col_start, int)
                        else slice(xt_col_start, xt_col_start + TT * BC),
                    ],
                )

        def emit_xfill_mm(xg_t, xts, m12):
            """One gate-chunk of the batched x-part into xg_t (+bias, bf16).

            Emitted in 256-col pieces, and the PSUM->SBUF bias-cast runs on
            the vector engine, so a mispredicted static schedule can only
            stall the PE/DVE queues briefly (never the ACT queue, which
            carries the recurrence's sigmoids/tanh)."""
            gate = "zrh"[m12 // 4]
            m = m12 % 4
            pxg = ps_xg.tile([128, TT * BC], fp32, tag="ps_xg", name="pxg")
            HP = TT * BC // 2
            for p in range(2):
                for k in range(KX):
                    nc.tensor.matmul(
                        pxg[:, p * HP : (p + 1) * HP],
                        wtile(gate, k, m),
                        xts[k][:, p * HP : (p + 1) * HP],
                        start=(k == 0),
                        stop=(k == KX - 1),
                        skip_group_check=True,
                    )
            for p in range(2):
                nc.vector.tensor_scalar_add(
                    xg_t[
                        :,
                        m12 * TT * BC + p * HP : m12 * TT * BC + (p + 1) * HP,
                    ],
                    pxg[:, p * HP : (p + 1) * HP],
                    bsb[:, m12 : m12 + 1],
                )

        def recurrence(xg_use, xg_fill, xt_use, xt_load, load_col_start,
                       hist_col_start):
            """TT steps using xg_use; interleave x-part fill of xg_fill from
            xt_use (DMA'd one recurrence earlier); prefetch the next tile's
            xT into xt_load at load_col_start."""
            xg3 = xg_use[:].rearrange("p (m s) -> p m s", m=12)
            hist = hist_pool.tile([128, TT * G * BC], bf16, tag="hist", name="hist")
            for s in range(TT):
                h_prev = h_cb[:] if s == 0 else hist[:, (s - 1) * 32 : s * 32]

                # padded to a full 2KB PSUM bank each, so pool slots never
                # share a bank (bank-overlap tracking would serialize ACT
                # reads of step s behind PE writes of step s+1)
                pr = ps_r.tile(
                    [128, 32], fp32, tag="ps_r", name="pr", padded_shape=[128, 512]
                )
                pz = ps_zr.tile(
                    [128, 32], fp32, tag="ps_z", name="pz", padded_shape=[128, 512]
                )
                pc = ps_c.tile(
                    [128, 32], fp32, tag="ps_c", name="pc", padded_shape=[128, 512]
                )
                # inject the precomputed x-parts (+bias)
                nc.tensor.matmul(
                    pr[:], ident[:], xg3[:, 4:8, s * BC : (s + 1) * BC],
                    start=True, stop=False, skip_group_check=True,
                )
                nc.tensor.matmul(
                    pz[:], ident[:], xg3[:, 0:4, s * BC : (s + 1) * BC],
                    start=True, stop=False, skip_group_check=True,
                )
                nc.tensor.matmul(
                    pc[:], ident[:], xg3[:, 8:12, s * BC : (s + 1) * BC],
                    start=True, stop=False, skip_group_check=True,
                )
                # r gate first: its 16 matmuls are the head of the serial chain
                r_sb = sm_pool.tile([128, 32], fp32, tag="r_sb", name="r_sb")
                rh_b = sm_pool.tile([128, 32], bf16, tag="rh_b", name="rh_b")
                for k in range(KH):
                    for m in range(4):
                        nc.tensor.matmul(
                            pr[:, m * 8 : (m + 1) * 8],
                            wtile("r", 2 + k, m),
                            h_prev[:, k * 8 : (k + 1) * 8],
                            start=False, stop=(k == KH - 1 and m == 3),
                            skip_group_check=True,
                        )
                nc.scalar.activation(
                    r_sb[:], pr[:], mybir.ActivationFunctionType.Sigmoid
                )
                # z gate fills the PE while sigmoid(r) and r*h run
                z_sb = sm_pool.tile([128, 32], fp32, tag="z_sb", name="z_sb")
                for k in range(KH):
                    for m in range(4):
                        nc.tensor.matmul(
                            pz[:, m * 8 : (m + 1) * 8],
                            wtile("z", 2 + k, m),
                            h_prev[:, k * 8 : (k + 1) * 8],
                            start=False, stop=(k == KH - 1 and m == 3),
                            skip_group_check=True,
                        )
                nc.scalar.activation(
                    z_sb[:], pz[:], mybir.ActivationFunctionType.Sigmoid
                )
                nc.vector.tensor_mul(rh_b[:], r_sb[:], h_prev[:])
                # candidate
                for k in range(KH):
                    for m in range(4):
                        nc.tensor.matmul(
                            pc[:, m * 8 : (m + 1) * 8],
                            wtile("h", 2 + k, m),
                            rh_b[:, k * 8 : (k + 1) * 8],
                            start=False, stop=(k == KH - 1 and m == 3),
                            skip_group_check=True,
                        )
                # vbar = (z-1)*h_prev, off the critical path
                vb_sb = sm_pool.tile([128, 32], fp32, tag="vb_sb", name="vb_sb")
                nc.vector.scalar_tensor_tensor(
                    vb_sb[:], z_sb[:], 1.0, h_prev[:],
                    mybir.AluOpType.subtract, mybir.AluOpType.mult,
                )
                c_sb = sm_pool.tile([128, 32], fp32, tag="c_sb", name="c_sb")
                nc.scalar.activation(
                    c_sb[:], pc[:], mybir.ActivationFunctionType.Tanh
                )
                # h_new = z*c - (z-1)*h, written straight into hist as bf16;
                # the hist slice doubles as next step's matmul rhs
                u_sb = sm_pool.tile([128, 32], fp32, tag="u_sb", name="u_sb")
                nc.vector.tensor_mul(u_sb[:], z_sb[:], c_sb[:])
                nc.vector.tensor_tensor(
                    hist[:, s * 32 : (s + 1) * 32], u_sb[:], vb_sb[:],
                    mybir.AluOpType.subtract,
                )
                if s == TT - 1:
                    # refresh the persistent carry for the next recurrence
                    nc.vector.tensor_tensor(
                        h_cb[:], u_sb[:], vb_sb[:], mybir.AluOpType.subtract
                    )

                # interleave the next tile's x-part work into PE/ACT gaps
                # (emitted at end-of-step so the ACT cast queues after tanh)
                if xg_fill is not None:
                    if s == 0:
                        emit_xfill_load(xt_load, load_col_start)
                    if s % 5 == 1 and s // 5 < 12:
                        emit_xfill_mm(xg_fill, xt_use, s // 5)

            nc.sync.dma_start(
                hist_d.ap()[
                    :,
                    bass.DynSlice(hist_col_start, TT * G * BC)
                    if not isinstance(hist_col_start, int)
                    else slice(hist_col_start, hist_col_start + TT * G * BC),
                ],
                hist[:],
            )

        # prologue: fill xga for tile 0 via xtB, then stage tile 1 in xtB
        emit_xfill_load(xtB, 0)
        for m12 in range(12):
            emit_xfill_mm(xga, xtB, m12)
        emit_xfill_load(xtB, TT * BC)

        CPB = TT * BC  # xT cols per tile
        HPB = TT * G * BC  # hist cols per tile
        # 4 tiles per loop body: halves the per-iteration back-edge cost
        # (engine barrier + semaphore resets + ACT table reload, ~7us each)
        assert n_tiles % 4 == 0
        with tc.For_i(
            0, n_tiles // 4, 1,
            hint_engines=tuple(mybir.ALL_ENGINES),
        ) as i:
            for j in range(4):
                use, fill = (xga, xgb) if j % 2 == 0 else (xgb, xga)
                xu, xl = (xtB, xtA) if j % 2 == 0 else (xtA, xtB)
                recurrence(
                    use, fill, xu, xl,
                    i * (4 * CPB) + (j + 2) * CPB,
                    i * (4 * HPB) + j * HPB,
                )

    nc.compile()
    return nc


def _run(inputs, n_tiles=T // TT, trace=False):
    from concourse.bass_utils import run_bass_kernel_spmd

    x = np.asarray(inputs["x"], dtype=np.float32)
    h0 = np.asarray(inputs["h0"], dtype=np.float32)
    Tn = n_tiles * TT
    x = x[:Tn]

    ws = {g: _prep_w(np.asarray(inputs[f"W{g}"], dtype=np.float32)) for g in "zrh"}
    bT = np.ascontiguousarray(
        np.stack(
            [np.asarray(inputs[f"b{g}"], dtype=np.float32).reshape(4, 128).T for g in "zrh"],
            axis=1,
        ).reshape(128, 12)
    )
    ident = np.eye(128, dtype=np.float32).astype(BF16)
    xT_all = x.astype(BF16).transpose(2, 0, 1)  # [D, Tn, B]

    in_maps = []
    for c in range(NCORES):
        sl = slice(c * BC, (c + 1) * BC)
        xT = np.zeros((D, (Tn + 2 * TT) * BC), dtype=BF16)
        xT[:, : Tn * BC] = xT_all[:, :, sl].reshape(D, Tn * BC)
        h0T = np.ascontiguousarray(
            h0[sl].reshape(BC, G, 128).transpose(2, 1, 0).reshape(128, G * BC)
        ).astype(BF16)
        in_maps.append(
            {
                "xT": xT,
                "h0T": h0T,
                "Wz": ws["z"], "Wr": ws["r"], "Wh": ws["h"],
                "bT": bT,
                "ident": ident,
            }
        )

    nc = _build_program(n_tiles)
    res = run_bass_kernel_spmd(nc, in_maps, core_ids=list(range(NCORES)), trace=trace)

    out = np.empty((Tn, B, H), dtype=np.float32)
    for c in range(NCORES):
        histT = res.results[c]["histT"]  # [128, Tn*G*BC] bf16
        out[:, c * BC : (c + 1) * BC, :] = (
            histT.reshape(128, Tn, G, BC).transpose(1, 3, 2, 0).reshape(Tn, BC, H)
            .astype(np.float32)
        )
    return out, res


def kernel(**inputs):
    out, _ = _run(inputs)
    return out


# revision 21
# speedup vs baseline: 1.2501x; 1.0528x over previous
"""GRU kernel for Trainium2 (Bass/Tile), 8-core batch-parallel.

Problem: x [T=2048, B=64, D=256] fp32, h0 [64, 512], Wz/Wr/Wh [768, 512],
bz/br/bh [512]. Returns hidden history [T, 64, 512] fp32.

Strategy:
  - Data-parallel over batch: core c handles batch rows c*8:(c+1)*8.
  - All on-device activations live in "transposed" layout: hidden dim on the
    128 partitions (4 column-groups of 8 batch cols), batch on the free dim.
  - Per timestep, the 3 gate matmuls run with the weight tile stationary
    (lhsT = W[k,m] 128x128 bf16) and hT streaming as an 8-column rhs.
  - The x-dependent part of all gates (x_t @ W[:D] + b) is batched over a
    64-step time tile as dense N=512 matmuls, cast to bf16 in SBUF, and
    injected into each step's PSUM accumulation via an identity matmul.
  - h is carried purely in bf16: the blend writes the hist tile directly
    (hist slice doubles as next step's matmul rhs), halving DVE work.
    The (1-z)*h term uses scalar_tensor_tensor: vbar=(z-1)*h; h'=u-vbar.
  - All 16 r-matmuls are emitted before the z-matmuls so sigmoid(r) fires
    as early as possible; z fills the PE while sigmoid(r)/r*h run.
  - xT tiles for the x-part are DMA'd one full 64-step tile ahead so the
    batched x-part matmuls never stall the PE queue on a DMA semaphore.
  - Host pre-transposes x to xT bf16 and post-transposes the output, so the
    device never transposes anything.
"""

import os
import sys

for _p in ("/opt/trn_rl_repo", os.path.expanduser("~/.axon_site/_ro/trn_rl_repo")):
    if os.path.isdir(_p) and _p not in sys.path:
        sys.path.insert(0, _p)

import numpy as np
import ml_dtypes

T, B, D, H = 2048, 64, 256, 512
NCORES = 8
BC = B // NCORES              # 8 batch rows per core
G = H // 128                  # 4 column-groups of the hidden dim
KH = H // 128                 # 4 contraction chunks for the h-part
KX = D // 128                 # 2 contraction chunks for the x-part
TT = 64                       # timesteps per loop iteration
BF16 = ml_dtypes.bfloat16


def _prep_w(w):
    # W [768, 512] -> [128, 6*512] bf16; col = k*512 + m*128 + j holds W[k*128+p, m*128+j]
    return np.ascontiguousarray(
        w.reshape(6, 128, 4, 128).transpose(1, 0, 2, 3).reshape(128, 3072)
    ).astype(BF16)


def _build_program(n_tiles):
    import concourse.bass as bass
    import concourse.tile as tile
    from concourse import bacc, mybir

    fp32 = mybir.dt.float32
    bf16 = mybir.dt.bfloat16
    Tn = n_tiles * TT

    nc = bacc.Bacc(
        "TRN2",
        target_bir_lowering=False,
        debug=False,
        enable_asserts=False,
        num_devices=NCORES,
    )

    assert n_tiles % 2 == 0, "loop body processes two tiles"
    # two tiles of zero padding at the end for the final (dead) prefetches
    xT_d = nc.dram_tensor("xT", [D, (Tn + 2 * TT) * BC], bf16, kind="ExternalInput")
    h0T_d = nc.dram_tensor("h0T", [128, G * BC], bf16, kind="ExternalInput")
    w_d = {
        g: nc.dram_tensor(f"W{g}", [128, 3072], bf16, kind="ExternalInput")
        for g in "zrh"
    }
    b_d = nc.dram_tensor("bT", [128, 12], fp32, kind="ExternalInput")
    id_d = nc.dram_tensor("ident", [128, 128], bf16, kind="ExternalInput")
    hist_d = nc.dram_tensor("histT", [128, Tn * G * BC], bf16, kind="ExternalOutput")

    from contextlib import ExitStack

    with tile.TileContext(nc) as tc, ExitStack() as ctx:
        persist = ctx.enter_context(tc.tile_pool(name="persist", bufs=1))
        wsb = {
            g: persist.tile([128, 3072], bf16, tag=f"W{g}", name=f"W{g}sb")
            for g in "zrh"
        }
        bsb = persist.tile([128, 12], fp32, tag="bT")
        ident = persist.tile([128, 128], bf16, tag="ident")
        h_cb = persist.tile([128, G * BC], bf16, tag="h_carry_b")   # h0 only

        for g in "zrh":
            nc.sync.dma_start(wsb[g][:], w_d[g].ap()[:])
        nc.sync.dma_start(bsb[:], b_d.ap()[:])
        nc.sync.dma_start(ident[:], id_d.ap()[:])
        nc.sync.dma_start(h_cb[:], h0T_d.ap()[:])

        xg_pool = ctx.enter_context(tc.tile_pool(name="xg", bufs=2))
        hist_pool = ctx.enter_context(tc.tile_pool(name="hist", bufs=2))
        sm_pool = ctx.enter_context(tc.tile_pool(name="small", bufs=3))
        # r-PSUM in two single-buffer banks so sigmoid(r) of the first two
        # output groups can fire while the last two groups' matmuls still
        # accumulate (PSUM dependency tracking is bank-granular); bufs=1 is
        # safe because step s+1's r-matmuls always start after sigmoid(s)
        ps_r1 = ctx.enter_context(tc.tile_pool(name="ps_r1", bufs=1, space="PSUM"))
        ps_r2 = ctx.enter_context(tc.tile_pool(name="ps_r2", bufs=1, space="PSUM"))
        ps_zr = ctx.enter_context(tc.tile_pool(name="ps_zr", bufs=2, space="PSUM"))
        ps_c = ctx.enter_context(tc.tile_pool(name="ps_c", bufs=2, space="PSUM"))
        ps_xg = ctx.enter_context(tc.tile_pool(name="ps_xg", bufs=2, space="PSUM"))

        xga = persist.tile([128, 12 * TT * BC], bf16, tag="xga")
        xgb = persist.tile([128, 12 * TT * BC], bf16, tag="xgb")
        # double-buffered persistent xT staging (one full tile ahead of use)
        xtA = [
            persist.tile([128, TT * BC], bf16, tag=f"xtA{k}", name=f"xtA{k}")
            for k in range(KX)
        ]
        xtB = [
            persist.tile([128, TT * BC], bf16, tag=f"xtB{k}", name=f"xtB{k}")
            for k in range(KX)
        ]

        def wtile(g, k, m):
            # lhsT tile for gate g, contraction chunk k (0,1=x-part, 2..5=h-part),
            # output chunk m
            return wsb[g][:, k * 512 + m * 128 : k * 512 + (m + 1) * 128]

        def emit_xfill_load(xts, xt_col_start):
            """DMA the xT slice for one future tile into persist tiles xts."""
            for k in range(KX):
                nc.sync.dma_start(
                    xts[k][:],
                    xT_d.ap()[
                        k * 128 : (k + 1) * 128,
                        bass.DynSlice(xt_col_start, TT * BC)
                        if not isinstance(xt_col_start, int)
                        else slice(xt_col_start, xt_col_start + TT * BC),
                    ],
                )

        def emit_xfill_mm(xg_t, xts, m12, gate_ap):
            """One gate-chunk of the batched x-part into xg_t (+bias, bf16).

            Emitted in 256-col pieces, and the PSUM->SBUF bias-cast runs on
            the vector engine, so a mispredicted static schedule can only
            stall the PE/DVE queues briefly (never the ACT queue, which
            carries the recurrence's sigmoids/tanh).

            gate_ap is a hist slice written near this chunk's intended step;
            the cast reads it via a bypass operand purely as a scheduling
            anchor, preventing the static scheduler from packing all twelve
            cast chains at the head of the tile (where they'd head-of-line
            block the recurrence's own DVE/PE queues)."""
            gate = "zrh"[m12 // 4]
            m = m12 % 4
            pxg = ps_xg.tile([128, TT * BC], fp32, tag="ps_xg", name="pxg")
            HP = TT * BC // 2
            for p in range(2):
                for k in range(KX):
                    nc.tensor.matmul(
                        pxg[:, p * HP : (p + 1) * HP],
                        wtile(gate, k, m),
                        xts[k][:, p * HP : (p + 1) * HP],
                        start=(k == 0),
                        stop=(k == KX - 1),
                        skip_group_check=True,
                    )
            for p in range(2):
                out_ap = xg_t[
                    :,
                    m12 * TT * BC + p * HP : m12 * TT * BC + (p + 1) * HP,
                ]
                if gate_ap is None:
                    nc.vector.tensor_scalar_add(
                        out_ap, pxg[:, p * HP : (p + 1) * HP], bsb[:, m12 : m12 + 1]
                    )
                else:
                    nc.vector.scalar_tensor_tensor(
                        out_ap,
                        pxg[:, p * HP : (p + 1) * HP],
                        bsb[:, m12 : m12 + 1],
                        gate_ap,
                        mybir.AluOpType.add,
                        mybir.AluOpType.bypass,
                    )

        def recurrence(xg_use, xg_fill, xt_use, xt_load, load_col_start,
                       hist_col_start):
            """TT steps using xg_use; interleave x-part fill of xg_fill from
            xt_use (DMA'd one recurrence earlier); prefetch the next tile's
            xT into xt_load at load_col_start."""
            xg3 = xg_use[:].rearrange("p (m s) -> p m s", m=12)
            hist = hist_pool.tile([128, TT * G * BC], bf16, tag="hist", name="hist")
            for s in range(TT):
                h_prev = h_cb[:] if s == 0 else hist[:, (s - 1) * 32 : s * 32]

                # padded to a full 2KB PSUM bank each, so pool slots never
                # share a bank (bank-overlap tracking would serialize ACT
                # reads of step s behind PE writes of step s+1)
                pr1 = ps_r1.tile(
                    [128, 16], fp32, tag="ps_r1", name="pr1", padded_shape=[128, 512]
                )
                pr2 = ps_r2.tile(
                    [128, 16], fp32, tag="ps_r2", name="pr2", padded_shape=[128, 512]
                )
                pz = ps_zr.tile(
                    [128, 32], fp32, tag="ps_z", name="pz", padded_shape=[128, 512]
                )
                pc = ps_c.tile(
                    [128, 32], fp32, tag="ps_c", name="pc", padded_shape=[128, 512]
                )
                # inject the precomputed x-parts (+bias)
                nc.tensor.matmul(
                    pr1[:], ident[:], xg3[:, 4:6, s * BC : (s + 1) * BC],
                    start=True, stop=False, skip_group_check=True,
                )
                nc.tensor.matmul(
                    pr2[:], ident[:], xg3[:, 6:8, s * BC : (s + 1) * BC],
                    start=True, stop=False, skip_group_check=True,
                )
                nc.tensor.matmul(
                    pz[:], ident[:], xg3[:, 0:4, s * BC : (s + 1) * BC],
                    start=True, stop=False, skip_group_check=True,
                )
                nc.tensor.matmul(
                    pc[:], ident[:], xg3[:, 8:12, s * BC : (s + 1) * BC],
                    start=True, stop=False, skip_group_check=True,
                )
                # r gate first, m-major in two bank-separate halves: sigmoid
                # of groups 0-1 fires while groups 2-3 still accumulate
                r_sb = sm_pool.tile([128, 32], fp32, tag="r_sb", name="r_sb")
                rh_b = sm_pool.tile([128, 32], bf16, tag="rh_b", name="rh_b")
                for m in range(4):
                    prh = pr1 if m < 2 else pr2
                    for k in range(KH):
                        nc.tensor.matmul(
                            prh[:, (m % 2) * 8 : (m % 2 + 1) * 8],
                            wtile("r", 2 + k, m),
                            h_prev[:, k * 8 : (k + 1) * 8],
                            start=False, stop=(m % 2 == 1 and k == KH - 1),
                            skip_group_check=True,
                        )
                    if m == 1:
                        nc.scalar.activation(
                            r_sb[:, 0:16], pr1[:],
                            mybir.ActivationFunctionType.Sigmoid,
                        )
                nc.scalar.activation(
                    r_sb[:, 16:32], pr2[:], mybir.ActivationFunctionType.Sigmoid
                )
                # z gate fills the PE while sigmoid(r) and r*h run
                z_sb = sm_pool.tile([128, 32], fp32, tag="z_sb", name="z_sb")
                for k in range(KH):
                    for m in range(4):
                        nc.tensor.matmul(
                            pz[:, m * 8 : (m + 1) * 8],
                            wtile("z", 2 + k, m),
                            h_prev[:, k * 8 : (k + 1) * 8],
                            start=False, stop=(k == KH - 1 and m == 3),
                            skip_group_check=True,
                        )
                nc.scalar.activation(
                    z_sb[:], pz[:], mybir.ActivationFunctionType.Sigmoid
                )
                nc.vector.tensor_mul(
                    rh_b[:, 0:16], r_sb[:, 0:16], h_prev[:, 0:16]
                )
                nc.vector.tensor_mul(
                    rh_b[:, 16:32], r_sb[:, 16:32], h_prev[:, 16:32]
                )
                # candidate
                for k in range(KH):
                    for m in range(4):
                        nc.tensor.matmul(
                            pc[:, m * 8 : (m + 1) * 8],
                            wtile("h", 2 + k, m),
                            rh_b[:, k * 8 : (k + 1) * 8],
                            start=False, stop=(k == KH - 1 and m == 3),
                            skip_group_check=True,
                        )
                # vbar = (z-1)*h_prev, off the critical path
                vb_sb = sm_pool.tile([128, 32], fp32, tag="vb_sb", name="vb_sb")
                nc.vector.scalar_tensor_tensor(
                    vb_sb[:], z_sb[:], 1.0, h_prev[:],
                    mybir.AluOpType.subtract, mybir.AluOpType.mult,
                )
                c_sb = sm_pool.tile([128, 32], fp32, tag="c_sb", name="c_sb")
                nc.scalar.activation(
                    c_sb[:], pc[:], mybir.ActivationFunctionType.Tanh
                )
                # h_new = z*c - (z-1)*h, written straight into hist as bf16;
                # the hist slice doubles as next step's matmul rhs
                u_sb = sm_pool.tile([128, 32], fp32, tag="u_sb", name="u_sb")
                nc.vector.tensor_mul(u_sb[:], z_sb[:], c_sb[:])
                nc.vector.tensor_tensor(
                    hist[:, s * 32 : (s + 1) * 32], u_sb[:], vb_sb[:],
                    mybir.AluOpType.subtract,
                )
                if s == TT - 1:
                    # refresh the persistent carry for the next recurrence
                    nc.vector.tensor_tensor(
                        h_cb[:], u_sb[:], vb_sb[:], mybir.AluOpType.subtract
                    )

                # interleave the next tile's x-part work into PE/ACT gaps
                # (emitted at end-of-step so the ACT cast queues after tanh)
                if xg_fill is not None:
                    if s == 0:
                        emit_xfill_load(xt_load, load_col_start)
                    if s % 5 == 1 and s // 5 < 12:
                        gs = max(0, s - 8) * 32
                        emit_xfill_mm(
                            xg_fill, xt_use, s // 5, hist[:, gs : gs + 256]
                        )

            nc.sync.dma_start(
                hist_d.ap()[
                    :,
                    bass.DynSlice(hist_col_start, TT * G * BC)
                    if not isinstance(hist_col_start, int)
                    else slice(hist_col_start, hist_col_start + TT * G * BC),
                ],
                hist[:],
            )

        # prologue: fill xga for tile 0 via xtB, then stage tile 1 in xtB
        emit_xfill_load(xtB, 0)
        for m12 in range(12):
            emit_xfill_mm(xga, xtB, m12, None)
        emit_xfill_load(xtB, TT * BC)

        CPB = TT * BC  # xT cols per tile
        HPB = TT * G * BC  # hist cols per tile
        # 4 tiles per loop body: halves the per-iteration back-edge cost
        # (engine barrier + semaphore resets + ACT table reload, ~7us each)
        assert n_tiles % 4 == 0
        with tc.For_i(
            0, n_tiles // 4, 1,
            hint_engines=tuple(mybir.ALL_ENGINES),
        ) as i:
            for j in range(4):
                use, fill = (xga, xgb) if j % 2 == 0 else (xgb, xga)
                xu, xl = (xtB, xtA) if j % 2 == 0 else (xtA, xtB)
                recurrence(
                    use, fill, xu, xl,
                    i * (4 * CPB) + (j + 2) * CPB,
                    i * (4 * HPB) + j * HPB,
                )

    nc.compile()
    return nc


def _run(inputs, n_tiles=T // TT, trace=False):
    from concourse.bass_utils import run_bass_kernel_spmd

    x = np.asarray(inputs["x"], dtype=np.float32)
    h0 = np.asarray(inputs["h0"], dtype=np.float32)
    Tn = n_tiles * TT
    x = x[:Tn]

    ws = {g: _prep_w(np.asarray(inputs[f"W{g}"], dtype=np.float32)) for g in "zrh"}
    bT = np.ascontiguousarray(
        np.stack(
            [np.asarray(inputs[f"b{g}"], dtype=np.float32).reshape(4, 128).T for g in "zrh"],
            axis=1,
        ).reshape(128, 12)
    )
    ident = np.eye(128, dtype=np.float32).astype(BF16)
    xT_all = x.astype(BF16).transpose(2, 0, 1)  # [D, Tn, B]

    in_maps = []
    for c in range(NCORES):
        sl = slice(c * BC, (c + 1) * BC)
        xT = np.zeros((D, (Tn + 2 * TT) * BC), dtype=BF16)
        xT[:, : Tn * BC] = xT_all[:, :, sl].reshape(D, Tn * BC)
        h0T = np.ascontiguousarray(
            h0[sl].reshape(BC, G, 128).transpose(2, 1, 0).reshape(128, G * BC)
        ).astype(BF16)
        in_maps.append(
            {
                "xT": xT,
                "h0T": h0T,
                "Wz": ws["z"], "Wr": ws["r"], "Wh": ws["h"],
                "bT": bT,
                "ident": ident,
            }
        )

    nc = _build_program(n_tiles)
    res = run_bass_kernel_spmd(nc, in_maps, core_ids=list(range(NCORES)), trace=trace)

    out = np.empty((Tn, B, H), dtype=np.float32)
    for c in range(NCORES):
        histT = res.results[c]["histT"]  # [128, Tn*G*BC] bf16
        out[:, c * BC : (c + 1) * BC, :] = (
            histT.reshape(128, Tn, G, BC).transpose(1, 3, 2, 0).reshape(Tn, BC, H)
            .astype(np.float32)
        )
    return out, res


def kernel(**inputs):
    out, _ = _run(inputs)
    return out


# revision 23
# speedup vs baseline: 1.2590x; 1.0071x over previous
"""GRU kernel for Trainium2 (Bass/Tile), 8-core batch-parallel.

Problem: x [T=2048, B=64, D=256] fp32, h0 [64, 512], Wz/Wr/Wh [768, 512],
bz/br/bh [512]. Returns hidden history [T, 64, 512] fp32.

Strategy:
  - Data-parallel over batch: core c handles batch rows c*8:(c+1)*8.
  - All on-device activations live in "transposed" layout: hidden dim on the
    128 partitions (4 column-groups of 8 batch cols), batch on the free dim.
  - Per timestep, the 3 gate matmuls run with the weight tile stationary
    (lhsT = W[k,m] 128x128 bf16) and hT streaming as an 8-column rhs.
  - The x-dependent part of all gates (x_t @ W[:D] + b) is batched over a
    64-step time tile as dense N=512 matmuls, cast to bf16 in SBUF, and
    injected into each step's PSUM accumulation via an identity matmul.
  - h is carried purely in bf16: the blend writes the hist tile directly
    (hist slice doubles as next step's matmul rhs), halving DVE work.
    The (1-z)*h term uses scalar_tensor_tensor: vbar=(z-1)*h; h'=u-vbar.
  - All 16 r-matmuls are emitted before the z-matmuls so sigmoid(r) fires
    as early as possible; z fills the PE while sigmoid(r)/r*h run.
  - xT tiles for the x-part are DMA'd one full 64-step tile ahead so the
    batched x-part matmuls never stall the PE queue on a DMA semaphore.
  - Host pre-transposes x to xT bf16 and post-transposes the output, so the
    device never transposes anything.
"""

import os
import sys

for _p in ("/opt/trn_rl_repo", os.path.expanduser("~/.axon_site/_ro/trn_rl_repo")):
    if os.path.isdir(_p) and _p not in sys.path:
        sys.path.insert(0, _p)

import numpy as np
import ml_dtypes

T, B, D, H = 2048, 64, 256, 512
NCORES = 8
BC = B // NCORES              # 8 batch rows per core
G = H // 128                  # 4 column-groups of the hidden dim
KH = H // 128                 # 4 contraction chunks for the h-part
KX = D // 128                 # 2 contraction chunks for the x-part
TT = 64                       # timesteps per loop iteration
BF16 = ml_dtypes.bfloat16


def _prep_w(w):
    # W [768, 512] -> [128, 6*512] bf16; col = k*512 + m*128 + j holds W[k*128+p, m*128+j]
    return np.ascontiguousarray(
        w.reshape(6, 128, 4, 128).transpose(1, 0, 2, 3).reshape(128, 3072)
    ).astype(BF16)


def _build_program(n_tiles):
    import concourse.bass as bass
    import concourse.tile as tile
    from concourse import bacc, mybir

    fp32 = mybir.dt.float32
    bf16 = mybir.dt.bfloat16
    Tn = n_tiles * TT

    nc = bacc.Bacc(
        "TRN2",
        target_bir_lowering=False,
        debug=False,
        enable_asserts=False,
        num_devices=NCORES,
    )

    assert n_tiles % 2 == 0, "loop body processes two tiles"
    # two tiles of zero padding at the end for the final (dead) prefetches
    xT_d = nc.dram_tensor("xT", [D, (Tn + 2 * TT) * BC], bf16, kind="ExternalInput")
    h0T_d = nc.dram_tensor("h0T", [128, G * BC], bf16, kind="ExternalInput")
    w_d = {
        g: nc.dram_tensor(f"W{g}", [128, 3072], bf16, kind="ExternalInput")
        for g in "zrh"
    }
    b_d = nc.dram_tensor("bT", [128, 12], fp32, kind="ExternalInput")
    id_d = nc.dram_tensor("ident", [128, 128], bf16, kind="ExternalInput")
    hist_d = nc.dram_tensor("histT", [128, Tn * G * BC], bf16, kind="ExternalOutput")

    from contextlib import ExitStack

    with tile.TileContext(nc) as tc, ExitStack() as ctx:
        persist = ctx.enter_context(tc.tile_pool(name="persist", bufs=1))
        wsb = {
            g: persist.tile([128, 3072], bf16, tag=f"W{g}", name=f"W{g}sb")
            for g in "zrh"
        }
        bsb = persist.tile([128, 12], fp32, tag="bT")
        ident = persist.tile([128, 128], bf16, tag="ident")
        h_cb = persist.tile([128, G * BC], bf16, tag="h_carry_b")   # h0 only

        for g in "zrh":
            nc.sync.dma_start(wsb[g][:], w_d[g].ap()[:])
        nc.sync.dma_start(bsb[:], b_d.ap()[:])
        nc.sync.dma_start(ident[:], id_d.ap()[:])
        nc.sync.dma_start(h_cb[:], h0T_d.ap()[:])

        xg_pool = ctx.enter_context(tc.tile_pool(name="xg", bufs=2))
        hist_pool = ctx.enter_context(tc.tile_pool(name="hist", bufs=2))
        sm_pool = ctx.enter_context(tc.tile_pool(name="small", bufs=3))
        # r-PSUM in two single-buffer banks so sigmoid(r) of the first two
        # output groups can fire while the last two groups' matmuls still
        # accumulate (PSUM dependency tracking is bank-granular); bufs=1 is
        # safe because step s+1's r-matmuls always start after sigmoid(s)
        ps_r1 = ctx.enter_context(tc.tile_pool(name="ps_r1", bufs=1, space="PSUM"))
        ps_r2 = ctx.enter_context(tc.tile_pool(name="ps_r2", bufs=1, space="PSUM"))
        # z/c PSUM single-buffered too (spine order guarantees the gate's
        # activation read of step s precedes step s+1's matmuls); the four
        # freed banks go to ps_xg, loosening the x-part fill chain
        ps_zr = ctx.enter_context(tc.tile_pool(name="ps_zr", bufs=1, space="PSUM"))
        ps_c = ctx.enter_context(tc.tile_pool(name="ps_c", bufs=1, space="PSUM"))
        ps_xg = ctx.enter_context(tc.tile_pool(name="ps_xg", bufs=4, space="PSUM"))

        xga = persist.tile([128, 12 * TT * BC], bf16, tag="xga")
        xgb = persist.tile([128, 12 * TT * BC], bf16, tag="xgb")
        # double-buffered persistent xT staging (one full tile ahead of use)
        xtA = [
            persist.tile([128, TT * BC], bf16, tag=f"xtA{k}", name=f"xtA{k}")
            for k in range(KX)
        ]
        xtB = [
            persist.tile([128, TT * BC], bf16, tag=f"xtB{k}", name=f"xtB{k}")
            for k in range(KX)
        ]

        def wtile(g, k, m):
            # lhsT tile for gate g, contraction chunk k (0,1=x-part, 2..5=h-part),
            # output chunk m
            return wsb[g][:, k * 512 + m * 128 : k * 512 + (m + 1) * 128]

        def emit_xfill_load(xts, xt_col_start):
            """DMA the xT slice for one future tile into persist tiles xts."""
            for k in range(KX):
                nc.sync.dma_start(
                    xts[k][:],
                    xT_d.ap()[
                        k * 128 : (k + 1) * 128,
                        bass.DynSlice(xt_col_start, TT * BC)
                        if not isinstance(xt_col_start, int)
                        else slice(xt_col_start, xt_col_start + TT * BC),
                    ],
                )

        def emit_xfill_mm(xg_t, xts, m12, gate_ap):
            """One gate-chunk of the batched x-part into xg_t (+bias, bf16).

            Emitted in 256-col pieces, and the PSUM->SBUF bias-cast runs on
            the vector engine, so a mispredicted static schedule can only
            stall the PE/DVE queues briefly (never the ACT queue, which
            carries the recurrence's sigmoids/tanh).

            gate_ap is a hist slice written near this chunk's intended step;
            the cast reads it via a bypass operand purely as a scheduling
            anchor, preventing the static scheduler from packing all twelve
            cast chains at the head of the tile (where they'd head-of-line
            block the recurrence's own DVE/PE queues)."""
            gate = "zrh"[m12 // 4]
            m = m12 % 4
            pxg = ps_xg.tile([128, TT * BC], fp32, tag="ps_xg", name="pxg")
            HP = TT * BC // 2
            for p in range(2):
                for k in range(KX):
                    nc.tensor.matmul(
                        pxg[:, p * HP : (p + 1) * HP],
                        wtile(gate, k, m),
                        xts[k][:, p * HP : (p + 1) * HP],
                        start=(k == 0),
                        stop=(k == KX - 1),
                        skip_group_check=True,
                    )
            for p in range(2):
                out_ap = xg_t[
                    :,
                    m12 * TT * BC + p * HP : m12 * TT * BC + (p + 1) * HP,
                ]
                if gate_ap is None:
                    nc.vector.tensor_scalar_add(
                        out_ap, pxg[:, p * HP : (p + 1) * HP], bsb[:, m12 : m12 + 1]
                    )
                else:
                    nc.vector.scalar_tensor_tensor(
                        out_ap,
                        pxg[:, p * HP : (p + 1) * HP],
                        bsb[:, m12 : m12 + 1],
                        gate_ap,
                        mybir.AluOpType.add,
                        mybir.AluOpType.bypass,
                    )

        def recurrence(xg_use, xg_fill, xt_use, xt_load, load_col_start,
                       hist_col_start):
            """TT steps using xg_use; interleave x-part fill of xg_fill from
            xt_use (DMA'd one recurrence earlier); prefetch the next tile's
            xT into xt_load at load_col_start."""
            xg3 = xg_use[:].rearrange("p (m s) -> p m s", m=12)
            hist = hist_pool.tile([128, TT * G * BC], bf16, tag="hist", name="hist")
            for s in range(TT):
                h_prev = h_cb[:] if s == 0 else hist[:, (s - 1) * 32 : s * 32]

                # padded to a full 2KB PSUM bank each, so pool slots never
                # share a bank (bank-overlap tracking would serialize ACT
                # reads of step s behind PE writes of step s+1)
                pr1 = ps_r1.tile(
                    [128, 16], fp32, tag="ps_r1", name="pr1", padded_shape=[128, 512]
                )
                pr2 = ps_r2.tile(
                    [128, 16], fp32, tag="ps_r2", name="pr2", padded_shape=[128, 512]
                )
                pz = ps_zr.tile(
                    [128, 32], fp32, tag="ps_z", name="pz", padded_shape=[128, 512]
                )
                pc = ps_c.tile(
                    [128, 32], fp32, tag="ps_c", name="pc", padded_shape=[128, 512]
                )
                # inject the precomputed x-parts (+bias)
                nc.tensor.matmul(
                    pr1[:], ident[:], xg3[:, 4:6, s * BC : (s + 1) * BC],
                    start=True, stop=False, skip_group_check=True,
                )
                nc.tensor.matmul(
                    pr2[:], ident[:], xg3[:, 6:8, s * BC : (s + 1) * BC],
                    start=True, stop=False, skip_group_check=True,
                )
                nc.tensor.matmul(
                    pz[:], ident[:], xg3[:, 0:4, s * BC : (s + 1) * BC],
                    start=True, stop=False, skip_group_check=True,
                )
                nc.tensor.matmul(
                    pc[:], ident[:], xg3[:, 8:12, s * BC : (s + 1) * BC],
                    start=True, stop=False, skip_group_check=True,
                )
                # r gate first, m-major in two bank-separate halves: sigmoid
                # of groups 0-1 fires while groups 2-3 still accumulate
                r_sb = sm_pool.tile([128, 32], fp32, tag="r_sb", name="r_sb")
                rh_b = sm_pool.tile([128, 32], bf16, tag="rh_b", name="rh_b")
                for m in range(4):
                    prh = pr1 if m < 2 else pr2
                    for k in range(KH):
                        nc.tensor.matmul(
                            prh[:, (m % 2) * 8 : (m % 2 + 1) * 8],
                            wtile("r", 2 + k, m),
                            h_prev[:, k * 8 : (k + 1) * 8],
                            start=False, stop=(m % 2 == 1 and k == KH - 1),
                            skip_group_check=True,
                        )
                    if m == 1:
                        nc.scalar.activation(
                            r_sb[:, 0:16], pr1[:],
                            mybir.ActivationFunctionType.Sigmoid,
                        )
                nc.scalar.activation(
                    r_sb[:, 16:32], pr2[:], mybir.ActivationFunctionType.Sigmoid
                )
                # z gate fills the PE while sigmoid(r) and r*h run
                z_sb = sm_pool.tile([128, 32], fp32, tag="z_sb", name="z_sb")
                for k in range(KH):
                    for m in range(4):
                        nc.tensor.matmul(
                            pz[:, m * 8 : (m + 1) * 8],
                            wtile("z", 2 + k, m),
                            h_prev[:, k * 8 : (k + 1) * 8],
                            start=False, stop=(k == KH - 1 and m == 3),
                            skip_group_check=True,
                        )
                nc.scalar.activation(
                    z_sb[:], pz[:], mybir.ActivationFunctionType.Sigmoid
                )
                nc.vector.tensor_mul(
                    rh_b[:, 0:16], r_sb[:, 0:16], h_prev[:, 0:16]
                )
                nc.vector.tensor_mul(
                    rh_b[:, 16:32], r_sb[:, 16:32], h_prev[:, 16:32]
                )
                # candidate
                for k in range(KH):
                    for m in range(4):
                        nc.tensor.matmul(
                            pc[:, m * 8 : (m + 1) * 8],
                            wtile("h", 2 + k, m),
                            rh_b[:, k * 8 : (k + 1) * 8],
                            start=False, stop=(k == KH - 1 and m == 3),
                            skip_group_check=True,
                        )
                # vbar = (z-1)*h_prev, off the critical path
                vb_sb = sm_pool.tile([128, 32], fp32, tag="vb_sb", name="vb_sb")
                nc.vector.scalar_tensor_tensor(
                    vb_sb[:], z_sb[:], 1.0, h_prev[:],
                    mybir.AluOpType.subtract, mybir.AluOpType.mult,
                )
                c_sb = sm_pool.tile([128, 32], fp32, tag="c_sb", name="c_sb")
                nc.scalar.activation(
                    c_sb[:], pc[:], mybir.ActivationFunctionType.Tanh
                )
                # h_new = z*c - (z-1)*h, written straight into hist as bf16;
                # the hist slice doubles as next step's matmul rhs
                u_sb = sm_pool.tile([128, 32], fp32, tag="u_sb", name="u_sb")
                nc.vector.tensor_mul(u_sb[:], z_sb[:], c_sb[:])
                nc.vector.tensor_tensor(
                    hist[:, s * 32 : (s + 1) * 32], u_sb[:], vb_sb[:],
                    mybir.AluOpType.subtract,
                )
                if s == TT - 1:
                    # refresh the persistent carry for the next recurrence
                    nc.vector.tensor_tensor(
                        h_cb[:], u_sb[:], vb_sb[:], mybir.AluOpType.subtract
                    )

                # interleave the next tile's x-part work into PE/ACT gaps
                # (emitted at end-of-step so the ACT cast queues after tanh)
                if xg_fill is not None:
                    if s == 0:
                        emit_xfill_load(xt_load, load_col_start)
                    if s % 5 == 1 and s // 5 < 12:
                        gs = max(0, s - 8) * 32
                        emit_xfill_mm(
                            xg_fill, xt_use, s // 5, hist[:, gs : gs + 256]
                        )

            nc.sync.dma_start(
                hist_d.ap()[
                    :,
                    bass.DynSlice(hist_col_start, TT * G * BC)
                    if not isinstance(hist_col_start, int)
                    else slice(hist_col_start, hist_col_start + TT * G * BC),
                ],
                hist[:],
            )

        # prologue: fill xga for tile 0 via xtB, then stage tile 1 in xtB
        emit_xfill_load(xtB, 0)
        for m12 in range(12):
            emit_xfill_mm(xga, xtB, m12, None)
        emit_xfill_load(xtB, TT * BC)

        CPB = TT * BC  # xT cols per tile
        HPB = TT * G * BC  # hist cols per tile
        # many tiles per loop body: amortizes the per-iteration back-edge
        # cost (engine barrier + semaphore resets + ACT table reload, ~7us)
        UF = 8 if n_tiles % 8 == 0 else 4
        assert n_tiles % UF == 0
        with tc.For_i(
            0, n_tiles // UF, 1,
            hint_engines=tuple(mybir.ALL_ENGINES),
        ) as i:
            for j in range(UF):
                use, fill = (xga, xgb) if j % 2 == 0 else (xgb, xga)
                xu, xl = (xtB, xtA) if j % 2 == 0 else (xtA, xtB)
                recurrence(
                    use, fill, xu, xl,
                    i * (UF * CPB) + (j + 2) * CPB,
                    i * (UF * HPB) + j * HPB,
                )

    nc.compile()
    return nc


def _run(inputs, n_tiles=T // TT, trace=False):
    from concourse.bass_utils import run_bass_kernel_spmd

    x = np.asarray(inputs["x"], dtype=np.float32)
    h0 = np.asarray(inputs["h0"], dtype=np.float32)
    Tn = n_tiles * TT
    x = x[:Tn]

    ws = {g: _prep_w(np.asarray(inputs[f"W{g}"], dtype=np.float32)) for g in "zrh"}
    bT = np.ascontiguousarray(
        np.stack(
            [np.asarray(inputs[f"b{g}"], dtype=np.float32).reshape(4, 128).T for g in "zrh"],
            axis=1,
        ).reshape(128, 12)
    )
    ident = np.eye(128, dtype=np.float32).astype(BF16)
    xT_all = x.astype(BF16).transpose(2, 0, 1)  # [D, Tn, B]

    in_maps = []
    for c in range(NCORES):
        sl = slice(c * BC, (c + 1) * BC)
        xT = np.zeros((D, (Tn + 2 * TT) * BC), dtype=BF16)
        xT[:, : Tn * BC] = xT_all[:, :, sl].reshape(D, Tn * BC)
        h0T = np.ascontiguousarray(
            h0[sl].reshape(BC, G, 128).transpose(2, 1, 0).reshape(128, G * BC)
        ).astype(BF16)
        in_maps.append(
            {
                "xT": xT,
                "h0T": h0T,
                "Wz": ws["z"], "Wr": ws["r"], "Wh": ws["h"],
                "bT": bT,
                "ident": ident,
            }
        )

    nc = _build_program(n_tiles)
    res = run_bass_kernel_spmd(nc, in_maps, core_ids=list(range(NCORES)), trace=trace)

    out = np.empty((Tn, B, H), dtype=np.float32)
    for c in range(NCORES):
        histT = res.results[c]["histT"]  # [128, Tn*G*BC] bf16
        out[:, c * BC : (c + 1) * BC, :] = (
            histT.reshape(128, Tn, G, BC).transpose(1, 3, 2, 0).reshape(Tn, BC, H)
            .astype(np.float32)
        )
    return out, res


def kernel(**inputs):
    out, _ = _run(inputs)
    return out
